# revision 1
# baseline (speedup 1.0000x reference)
"""Trainium2 Bass kernel for nn_CycleEmbedding0 (gnn_message_passing).

Computes out = segment_sum(emb_W[x][atom_to_cycle[0]], atom_to_cycle[1], 200000).

Key algebraic reduction: the embedding table has only VOCAB=22 rows, so
    out[c, :] = sum_v H[c, v] * emb_W[v, :]
where H[c, v] = #{pairs p : seg[p] == c and x[src[p]] == v} is a class
histogram.  This cuts memory traffic ~8x vs the naive gather/scatter.

Distribution (8 NeuronCores): cycle bins are range-sharded across cores
(25000 bins/core).  On the host, each core's bins are load-balanced into
440 windows of 64 bin-slots (serpentine assignment by bin popularity) and
the core's pairs are bucketed per window, padded to C chunks of 128 pairs.

Device kernel per core (identical SPMD program):
  stage 1 (histogram): per 128-pair chunk, VectorE builds a one-hot
    OH[p, slot] = (iota64 == slot_of_pair) with a single 4x-mode
    tensor_scalar; TensorE accumulates psum_HT[22, 64] += OC^T @ OH where
    OC = per-pair class one-hots (pre-built on host, streamed in as bf16).
  stage 2 (apply emb): per window-pair, out[128, 128] = HT^T @ (W_hi+W_lo)
    with emb_W split into two bf16 matrices for fp32-level accuracy;
    ScalarE evacuates PSUM; DMA writes [28160, 128] f32 per core.

Host gathers the 8 core outputs and un-permutes rows back to cycle order.
"""

import numpy as np
import ml_dtypes
from contextlib import ExitStack

import concourse.bass as bass
import concourse.tile as tile
import concourse.mybir as mybir
from concourse import bacc
from concourse.bass_utils import run_bass_kernel_spmd

BF16 = ml_dtypes.bfloat16

N_ATOMS = 500000
N_PAIRS = 2000000
N_CYCLES = 200000
VOCAB = 22
HIDDEN = 128

NCORES = 8
BPC = N_CYCLES // NCORES      # bins (cycles) per core
W = 64                        # bin-slots per window
NWIN = 440                    # windows per core
NBLK = 8                      # OC is DMA'd in NBLK blocks
WPB = NWIN // NBLK            # windows per block
GROUP = 4                     # windows per psum group
SMAX = -(-BPC // NWIN)        # max slot index + 1 (<= 64)
assert SMAX <= W and NWIN % GROUP == 0 and NWIN % NBLK == 0

_prog_cache: dict = {}


def _build_program(C: int):
    """One SPMD program, parameterized by C = chunks per window."""
    NCH = NWIN * C
    nc = bacc.Bacc("TRN2", target_bir_lowering=False, debug=False,
                   num_devices=NCORES)
    iota_d = nc.dram_tensor("iota", [128, W], mybir.dt.bfloat16,
                            kind="ExternalInput")
    wmat_d = nc.dram_tensor("wmat", [VOCAB, 2 * HIDDEN], mybir.dt.bfloat16,
                            kind="ExternalInput")
    binv_d = nc.dram_tensor("binv", [128, NCH], mybir.dt.float32,
                            kind="ExternalInput")
    oc_d = nc.dram_tensor("oc", [128, NCH * VOCAB], mybir.dt.bfloat16,
                          kind="ExternalInput")
    out_d = nc.dram_tensor("out", [NWIN * W, HIDDEN], mybir.dt.float32,
                           kind="ExternalOutput")
    out_ap = out_d.ap()

    with tile.TileContext(nc) as tc:
        with ExitStack() as ctx:
            const = ctx.enter_context(tc.tile_pool(name="const", bufs=1))
            ocpool = ctx.enter_context(tc.tile_pool(name="ocblk", bufs=NBLK))
            ohpool = ctx.enter_context(tc.tile_pool(name="oh", bufs=8))
            htpool = ctx.enter_context(tc.tile_pool(name="hts", bufs=3))
            outpool = ctx.enter_context(tc.tile_pool(name="outs", bufs=3))
            ps_ht = ctx.enter_context(
                tc.tile_pool(name="psht", bufs=2, space=bass.MemorySpace.PSUM))
            ps_out = ctx.enter_context(
                tc.tile_pool(name="psout", bufs=2, space=bass.MemorySpace.PSUM))

            iota = const.tile([128, W], mybir.dt.bfloat16)
            nc.default_dma_engine.dma_start(iota[:], iota_d.ap())
            wmat = const.tile([VOCAB, 2 * HIDDEN], mybir.dt.bfloat16)
            nc.default_dma_engine.dma_start(wmat[:], wmat_d.ap())
            binv = const.tile([128, NCH], mybir.dt.float32)
            nc.default_dma_engine.dma_start(binv[:], binv_d.ap())

            occols = WPB * C * VOCAB
            ocblk = []
            for b in range(NBLK):
                t = ocpool.tile([128, occols], mybir.dt.bfloat16)
                nc.default_dma_engine.dma_start(
                    t[:], oc_d.ap()[:, b * occols:(b + 1) * occols])
                ocblk.append(t)

            for g in range(NWIN // GROUP):
                ht = ps_ht.tile([VOCAB, GROUP * W], mybir.dt.float32)
                for wi in range(GROUP):
                    w = g * GROUP + wi
                    blk, wloc = divmod(w, WPB)
                    for j in range(C):
                        i = w * C + j
                        oh = ohpool.tile([128, W], mybir.dt.bfloat16)
                        nc.vector.tensor_scalar(
                            oh[:], iota[:], binv[:, i:i + 1], None,
                            mybir.AluOpType.is_equal)
                        col = (wloc * C + j) * VOCAB
                        nc.tensor.matmul(
                            ht[:, wi * W:(wi + 1) * W],
                            ocblk[blk][:, col:col + VOCAB],
                            oh[:],
                            start=(j == 0), stop=(j == C - 1))
                hts = htpool.tile([VOCAB, GROUP * W], mybir.dt.bfloat16)
                nc.scalar.copy(hts[:], ht[:])
                ops = ps_out.tile([128, 2 * HIDDEN], mybir.dt.float32)
                for wp in range(2):
                    lhsT = hts[:, wp * 128:(wp + 1) * 128]
                    o = ops[:, wp * HIDDEN:(wp + 1) * HIDDEN]
                    nc.tensor.matmul(o, lhsT, wmat[:, 0:HIDDEN],
                                     start=True, stop=False)
                    nc.tensor.matmul(o, lhsT, wmat[:, HIDDEN:2 * HIDDEN],
                                     start=False, stop=True)
                outs = outpool.tile([128, 2 * HIDDEN], mybir.dt.float32)
                nc.scalar.copy(outs[:], ops[:])
                for wp in range(2):
                    r0 = (g * GROUP + 2 * wp) * W
                    nc.default_dma_engine.dma_start(
                        out_ap[r0:r0 + 128, :],
                        outs[:, wp * HIDDEN:(wp + 1) * HIDDEN])
    nc.compile()
    return nc


def _prep_core(local: np.ndarray, cls: np.ndarray, C: int | None):
    """Window-balance one core's pairs.  Returns (wcnt_max) when C is None,
    else (binv[128,NCH] f32, oc[128,NCH*22] bf16, row_of_local[BPC])."""
    cnt = np.bincount(local, minlength=BPC)
    order = np.argsort(cnt, kind="stable")[::-1]
    r = np.arange(BPC)
    passi, pos = divmod(r, NWIN)
    wser = np.where(passi % 2 == 0, pos, NWIN - 1 - pos)
    w_of_bin = np.empty(BPC, np.int32)
    s_of_bin = np.empty(BPC, np.int32)
    w_of_bin[order] = wser
    s_of_bin[order] = passi
    wkey = w_of_bin[local]
    wcnt = np.bincount(wkey, minlength=NWIN)
    if C is None:
        return int(wcnt.max())

    NCH = NWIN * C
    order1 = np.argsort(wkey, kind="stable")
    wsorted = wkey[order1]
    starts = np.zeros(NWIN, np.int64)
    np.cumsum(wcnt[:-1], out=starts[1:])
    idx_in_w = np.arange(len(local)) - starts[wsorted]
    dest = wsorted.astype(np.int64) * (C * 128) + idx_in_w

    binv_pad = np.full(NCH * 128, -1.0, np.float32)
    binv_pad[dest] = s_of_bin[local[order1]]
    cls_pad = np.full(NCH * 128, VOCAB, np.int16)
    cls_pad[dest] = cls[order1]

    eye = np.zeros((VOCAB + 1, VOCAB), BF16)
    eye[np.arange(VOCAB), np.arange(VOCAB)] = 1
    binv_in = np.ascontiguousarray(binv_pad.reshape(NCH, 128).T)
    oc_in = np.ascontiguousarray(
        eye[cls_pad].reshape(NCH, 128, VOCAB).transpose(1, 0, 2)
    ).reshape(128, NCH * VOCAB)
    row_of_local = (w_of_bin * W + s_of_bin).astype(np.int64)
    return binv_in, oc_in, row_of_local


def _make_in_maps(x, atom_to_cycle, emb_W, C=None):
    src = np.asarray(atom_to_cycle[0], dtype=np.int64)
    seg = np.asarray(atom_to_cycle[1], dtype=np.int64)
    cls_all = np.asarray(x, dtype=np.int16)[src]

    order0 = np.argsort(seg, kind="stable")
    seg_s = seg[order0]
    cls_s = cls_all[order0]
    bounds = np.searchsorted(seg_s, np.arange(NCORES + 1) * BPC)

    cores = []
    for c in range(NCORES):
        lo, hi = bounds[c], bounds[c + 1]
        cores.append((np.asarray(seg_s[lo:hi] - c * BPC, np.int64),
                      cls_s[lo:hi]))

    if C is None:
        C = max(5, -(-max(_prep_core(l, k, None) for l, k in cores) // 128))

    iota_in = np.broadcast_to(
        np.arange(W, dtype=np.float32), (128, W)).astype(BF16).copy()
    w32 = np.asarray(emb_W, np.float32)
    w_hi = w32.astype(BF16)
    w_lo = (w32 - w_hi.astype(np.float32)).astype(BF16)
    wmat_in = np.concatenate([w_hi, w_lo], axis=1)

    in_maps, rowmaps = [], []
    for local, k in cores:
        binv_in, oc_in, rowmap = _prep_core(local, k, C)
        in_maps.append({"iota": iota_in, "wmat": wmat_in,
                        "binv": binv_in, "oc": oc_in})
        rowmaps.append(rowmap)
    return C, in_maps, rowmaps


def kernel(x, atom_to_cycle, emb_W, n_cycles):
    assert int(n_cycles) == N_CYCLES
    x = np.asarray(x)
    atom_to_cycle = np.asarray(atom_to_cycle)
    emb_W = np.asarray(emb_W, np.float32)
    assert atom_to_cycle.shape == (2, N_PAIRS) and emb_W.shape == (VOCAB, HIDDEN)

    C, in_maps, rowmaps = _make_in_maps(x, atom_to_cycle, emb_W)
    if C not in _prog_cache:
        _prog_cache[C] = _build_program(C)
    nc = _prog_cache[C]

    res = run_bass_kernel_spmd(nc, in_maps, list(range(NCORES))).results

    out = np.empty((N_CYCLES, HIDDEN), np.float32)
    for c in range(NCORES):
        out[c * BPC:(c + 1) * BPC] = res[c]["out"][rowmaps[c]]
    return out


# revision 2
# speedup vs baseline: 1.4374x; 1.4374x over previous
"""Trainium2 Bass kernel for nn_CycleEmbedding0 (gnn_message_passing).

Computes out = segment_sum(emb_W[x][atom_to_cycle[0]], atom_to_cycle[1], 200000).

Key algebraic reduction: the embedding table has only VOCAB=22 rows, so
    out[c, :] = sum_v H[c, v] * emb_W[v, :]
where H[c, v] = #{pairs p : seg[p] == c and x[src[p]] == v} is a class
histogram.  This cuts memory traffic ~8x vs the naive gather/scatter.

Distribution (8 NeuronCores): cycle bins are range-sharded across cores
(25000 bins/core).  On the host, each core's bins are load-balanced into
392 windows of 64 bin-slots (serpentine assignment by bin popularity) and
the core's pairs are bucketed per window, padded to C chunks of 128.

Device kernel per core (identical SPMD program):
  stage 1 (histogram): per 256-pair double-chunk, TensorE accumulates
    psum_HT[v, slot] += sum_i OC[:,i,:].T @ OH[:,i,:] with fp8 DoubleRow
    matmuls (2 MACs/cell/cycle).  OH (slot one-hots) and OC (class
    one-hots) are built on the host as fp8 and streamed in.
  stage 2 (apply emb): per window-pair, out[128, 128] = HT^T @ (W_hi+W_lo)
    with emb_W split into two bf16 matrices for fp32-level accuracy;
    ScalarE evacuates the histogram, VectorE evacuates the output;
    DMA writes [25088, 128] f32 per core.

Host gathers the 8 core outputs and un-permutes rows back to cycle order.
"""

import numpy as np
import ml_dtypes
from contextlib import ExitStack

import concourse.bass as bass
import concourse.tile as tile
import concourse.mybir as mybir
from concourse import bacc
from concourse.bass_utils import run_bass_kernel_spmd

BF16 = ml_dtypes.bfloat16
FP8 = ml_dtypes.float8_e4m3

N_ATOMS = 500000
N_PAIRS = 2000000
N_CYCLES = 200000
VOCAB = 22
HIDDEN = 128

NCORES = 8
BPC = N_CYCLES // NCORES      # bins (cycles) per core
W = 64                        # bin-slots per window
VC = 32                       # class cols padded (DoubleRow needs step%16==0)
NWIN = 392                    # windows per core
NBLK = 8                      # OH/OC streamed in NBLK blocks
WPB = NWIN // NBLK            # windows per block
GROUP = 4                     # windows per psum group
SMAX = -(-BPC // NWIN)        # max slot index + 1 (<= 64)
assert SMAX <= W and NWIN % GROUP == 0 and NWIN % NBLK == 0

_prog_cache: dict = {}


def _build_program(C: int):
    """One SPMD program; C (even) chunks of 128 pairs per window."""
    assert C % 2 == 0
    D = C // 2                    # double-chunks per window
    NCH2 = NWIN * D               # double-chunks per core
    nc = bacc.Bacc("TRN2", target_bir_lowering=False, debug=False,
                   num_devices=NCORES)
    wmat_d = nc.dram_tensor("wmat", [VOCAB, 2 * HIDDEN], mybir.dt.bfloat16,
                            kind="ExternalInput")
    oh_d = nc.dram_tensor("oh", [128, NCH2 * 2 * W], mybir.dt.float8e4,
                          kind="ExternalInput")
    oc_d = nc.dram_tensor("oc", [128, NCH2 * 2 * VC], mybir.dt.float8e4,
                          kind="ExternalInput")
    out_d = nc.dram_tensor("out", [NWIN * W, HIDDEN], mybir.dt.float32,
                           kind="ExternalOutput")
    out_ap = out_d.ap()
    ohcols = WPB * D * 2 * W      # per-block free dim
    occols = WPB * D * 2 * VC

    with tile.TileContext(nc) as tc:
        with ExitStack() as ctx:
            const = ctx.enter_context(tc.tile_pool(name="const", bufs=1))
            ohpool = ctx.enter_context(tc.tile_pool(name="ohblk", bufs=4))
            ocpool = ctx.enter_context(tc.tile_pool(name="ocblk", bufs=4))
            htpool = ctx.enter_context(tc.tile_pool(name="hts", bufs=3))
            outpool = ctx.enter_context(tc.tile_pool(name="outs", bufs=3))
            ps_ht = ctx.enter_context(
                tc.tile_pool(name="psht", bufs=2, space=bass.MemorySpace.PSUM))
            ps_out = ctx.enter_context(
                tc.tile_pool(name="psout", bufs=2, space=bass.MemorySpace.PSUM))

            wmat = const.tile([VOCAB, 2 * HIDDEN], mybir.dt.bfloat16)
            nc.default_dma_engine.dma_start(wmat[:], wmat_d.ap())

            oh_t: dict = {}
            oc_t: dict = {}
            for g in range(NWIN // GROUP):
                ht = ps_ht.tile([VC, GROUP * W], mybir.dt.float32)
                for wi in range(GROUP):
                    w = g * GROUP + wi
                    blk, wloc = divmod(w, WPB)
                    if wloc == 0:
                        t = ohpool.tile([128, ohcols], mybir.dt.float8e4)
                        nc.default_dma_engine.dma_start(
                            t[:], oh_d.ap()[:, blk * ohcols:(blk + 1) * ohcols])
                        oh_t[blk] = t
                        t = ocpool.tile([128, occols], mybir.dt.float8e4)
                        nc.default_dma_engine.dma_start(
                            t[:], oc_d.ap()[:, blk * occols:(blk + 1) * occols])
                        oc_t[blk] = t
                    for dc in range(D):
                        j = wloc * D + dc
                        oh3 = oh_t[blk][:, j * 2 * W:(j + 1) * 2 * W].rearrange(
                            "p (two s) -> p two s", two=2)
                        oc3 = oc_t[blk][:, j * 2 * VC:(j + 1) * 2 * VC].rearrange(
                            "p (two v) -> p two v", two=2)
                        nc.tensor.matmul(
                            ht[:, wi * W:(wi + 1) * W], oc3, oh3,
                            start=(dc == 0), stop=(dc == D - 1),
                            perf_mode=mybir.MatmulPerfMode.DoubleRow)
                hts = htpool.tile([VOCAB, GROUP * W], mybir.dt.bfloat16)
                nc.scalar.copy(hts[:], ht[0:VOCAB, :])
                ops = ps_out.tile([128, 2 * HIDDEN], mybir.dt.float32)
                for wp in range(2):
                    lhsT = hts[:, wp * 128:(wp + 1) * 128]
                    o = ops[:, wp * HIDDEN:(wp + 1) * HIDDEN]
                    nc.tensor.matmul(o, lhsT, wmat[:, 0:HIDDEN],
                                     start=True, stop=False)
                    nc.tensor.matmul(o, lhsT, wmat[:, HIDDEN:2 * HIDDEN],
                                     start=False, stop=True)
                outs = outpool.tile([128, 2 * HIDDEN], mybir.dt.float32)
                nc.vector.tensor_copy(outs[:], ops[:])
                for wp in range(2):
                    r0 = (g * GROUP + 2 * wp) * W
                    nc.default_dma_engine.dma_start(
                        out_ap[r0:r0 + 128, :],
                        outs[:, wp * HIDDEN:(wp + 1) * HIDDEN])
    nc.compile()
    return nc


_EYE_OH = np.zeros((W + 1, W), FP8)
_EYE_OH[np.arange(W), np.arange(W)] = 1
_EYE_OC = np.zeros((VOCAB + 1, VC), FP8)
_EYE_OC[np.arange(VOCAB), np.arange(VOCAB)] = 1


def _prep_core(local: np.ndarray, cls: np.ndarray, C: int | None):
    """Window-balance one core's pairs.  Returns wcnt_max when C is None,
    else (oh[128, NCH2*128] fp8, oc[128, NCH2*64] fp8, row_of_local[BPC])."""
    cnt = np.bincount(local, minlength=BPC)
    order = np.argsort(cnt, kind="stable")[::-1]
    r = np.arange(BPC)
    passi, pos = divmod(r, NWIN)
    wser = np.where(passi % 2 == 0, pos, NWIN - 1 - pos)
    w_of_bin = np.empty(BPC, np.int32)
    s_of_bin = np.empty(BPC, np.int32)
    w_of_bin[order] = wser
    s_of_bin[order] = passi
    wkey = w_of_bin[local]
    wcnt = np.bincount(wkey, minlength=NWIN)
    if C is None:
        return int(wcnt.max())

    NCH2 = NWIN * C // 2
    order1 = np.argsort(wkey, kind="stable")
    wsorted = wkey[order1]
    starts = np.zeros(NWIN, np.int64)
    np.cumsum(wcnt[:-1], out=starts[1:])
    idx_in_w = np.arange(len(local)) - starts[wsorted]
    dest = wsorted.astype(np.int64) * (C * 128) + idx_in_w

    slot_pad = np.full(NWIN * C * 128, W, np.int16)
    slot_pad[dest] = s_of_bin[local[order1]]
    cls_pad = np.full(NWIN * C * 128, VOCAB, np.int16)
    cls_pad[dest] = cls[order1]

    oh_in = np.ascontiguousarray(
        _EYE_OH[slot_pad].reshape(NCH2, 2, 128, W).transpose(2, 0, 1, 3)
    ).reshape(128, NCH2 * 2 * W)
    oc_in = np.ascontiguousarray(
        _EYE_OC[cls_pad].reshape(NCH2, 2, 128, VC).transpose(2, 0, 1, 3)
    ).reshape(128, NCH2 * 2 * VC)
    row_of_local = (w_of_bin * W + s_of_bin).astype(np.int64)
    return oh_in, oc_in, row_of_local


def _make_in_maps(x, atom_to_cycle, emb_W, C=None):
    src = np.asarray(atom_to_cycle[0], dtype=np.int64)
    seg = np.asarray(atom_to_cycle[1], dtype=np.int64)
    cls_all = np.asarray(x, dtype=np.int16)[src]

    order0 = np.argsort(seg, kind="stable")
    seg_s = seg[order0]
    cls_s = cls_all[order0]
    bounds = np.searchsorted(seg_s, np.arange(NCORES + 1) * BPC)

    cores = []
    for c in range(NCORES):
        lo, hi = bounds[c], bounds[c + 1]
        cores.append((np.asarray(seg_s[lo:hi] - c * BPC, np.int64),
                      cls_s[lo:hi]))

    if C is None:
        wmax = max(_prep_core(l, k, None) for l, k in cores)
        C = max(6, 2 * (-(-wmax // 256)))

    w32 = np.asarray(emb_W, np.float32)
    w_hi = w32.astype(BF16)
    w_lo = (w32 - w_hi.astype(np.float32)).astype(BF16)
    wmat_in = np.concatenate([w_hi, w_lo], axis=1)

    in_maps, rowmaps = [], []
    for local, k in cores:
        oh_in, oc_in, rowmap = _prep_core(local, k, C)
        in_maps.append({"wmat": wmat_in, "oh": oh_in, "oc": oc_in})
        rowmaps.append(rowmap)
    return C, in_maps, rowmaps


def kernel(x, atom_to_cycle, emb_W, n_cycles):
    assert int(n_cycles) == N_CYCLES
    x = np.asarray(x)
    atom_to_cycle = np.asarray(atom_to_cycle)
    emb_W = np.asarray(emb_W, np.float32)
    assert atom_to_cycle.shape == (2, N_PAIRS) and emb_W.shape == (VOCAB, HIDDEN)

    C, in_maps, rowmaps = _make_in_maps(x, atom_to_cycle, emb_W)
    if C not in _prog_cache:
        _prog_cache[C] = _build_program(C)
    nc = _prog_cache[C]

    res = run_bass_kernel_spmd(nc, in_maps, list(range(NCORES))).results

    out = np.empty((N_CYCLES, HIDDEN), np.float32)
    for c in range(NCORES):
        out[c * BPC:(c + 1) * BPC] = res[c]["out"][rowmaps[c]]
    return out


# revision 4
# speedup vs baseline: 1.6825x; 1.1705x over previous
"""Trainium2 Bass kernel for nn_CycleEmbedding0 (gnn_message_passing).

Computes out = segment_sum(emb_W[x][atom_to_cycle[0]], atom_to_cycle[1], 200000).

Key algebraic reduction: the embedding table has only VOCAB=22 rows, so
    out[c, :] = sum_v H[c, v] * emb_W[v, :]
where H[c, v] = #{pairs p : seg[p] == c and x[src[p]] == v} is a class
histogram.  This cuts memory traffic ~8x vs the naive gather/scatter.

Distribution (8 NeuronCores): cycle bins are range-sharded across cores
(25000 bins/core).  On the host, each core's bins are load-balanced into
392 windows of 64 bin-slots (serpentine assignment by bin popularity) and
the core's pairs are bucketed per window, padded to C chunks of 128.

Device kernel per core (identical SPMD program):
  stage 1 (histogram): per 256-pair double-chunk, TensorE accumulates
    psum_HT[v, slot] += sum_i OC[:,i,:].T @ OH[:,i,:] with fp8 DoubleRow
    matmuls (2 MACs/cell/cycle).  OH (slot one-hots) and OC (class
    one-hots) are built on the host as fp8 and streamed in.
  stage 2 (apply emb): per window-pair, out[128, 128] = HT^T @ (W_hi+W_lo)
    with emb_W split into two bf16 matrices for fp32-level accuracy;
    ScalarE evacuates the histogram, VectorE evacuates the output;
    DMA writes [25088, 128] f32 per core.

Host gathers the 8 core outputs and un-permutes rows back to cycle order.
"""

import numpy as np
import ml_dtypes
from contextlib import ExitStack

import concourse.bass as bass
import concourse.tile as tile
import concourse.mybir as mybir
from concourse import bacc
from concourse.bass_utils import run_bass_kernel_spmd

BF16 = ml_dtypes.bfloat16
FP8 = ml_dtypes.float8_e4m3

N_ATOMS = 500000
N_PAIRS = 2000000
N_CYCLES = 200000
VOCAB = 22
HIDDEN = 128

NCORES = 8
BPC = N_CYCLES // NCORES      # bins (cycles) per core
W = 64                        # bin-slots per window
VC = 32                       # class cols padded (DoubleRow needs step%16==0)
NWIN = 392                    # windows per core
NBLK = 8                      # OH/OC streamed in NBLK blocks
WPB = NWIN // NBLK            # windows per block
GROUP = 4                     # windows per psum group
SMAX = -(-BPC // NWIN)        # max slot index + 1 (<= 64)
assert SMAX <= W and NWIN % GROUP == 0 and NWIN % NBLK == 0

_prog_cache: dict = {}


def _build_program(C: int):
    """One SPMD program; C (even) chunks of 128 pairs per window."""
    assert C % 2 == 0
    D = C // 2                    # double-chunks per window
    NCH2 = NWIN * D               # double-chunks per core
    nc = bacc.Bacc("TRN2", target_bir_lowering=False, debug=False,
                   num_devices=NCORES)
    wmat_d = nc.dram_tensor("wmat", [VOCAB, 2 * HIDDEN], mybir.dt.bfloat16,
                            kind="ExternalInput")
    oh_d = nc.dram_tensor("oh", [128, NCH2 * 2 * W], mybir.dt.float8e4,
                          kind="ExternalInput")
    oc_d = nc.dram_tensor("oc", [128, NCH2 * 2 * VC], mybir.dt.float8e4,
                          kind="ExternalInput")
    out_d = nc.dram_tensor("out", [NWIN * W, HIDDEN], mybir.dt.float32,
                           kind="ExternalOutput")
    out_ap = out_d.ap()
    ohcols = WPB * D * 2 * W      # per-block free dim
    occols = WPB * D * 2 * VC

    with tile.TileContext(nc) as tc:
        with ExitStack() as ctx:
            const = ctx.enter_context(tc.tile_pool(name="const", bufs=1))
            ohpool = ctx.enter_context(tc.tile_pool(name="ohblk", bufs=4))
            ocpool = ctx.enter_context(tc.tile_pool(name="ocblk", bufs=4))
            htpool = ctx.enter_context(tc.tile_pool(name="hts", bufs=3))
            outpool = ctx.enter_context(tc.tile_pool(name="outs", bufs=3))
            ps_ht = ctx.enter_context(
                tc.tile_pool(name="psht", bufs=3, space=bass.MemorySpace.PSUM))
            ps_out = ctx.enter_context(
                tc.tile_pool(name="psout", bufs=2, space=bass.MemorySpace.PSUM))

            wmat = const.tile([VOCAB, 2 * HIDDEN], mybir.dt.bfloat16)
            nc.default_dma_engine.dma_start(wmat[:], wmat_d.ap())

            oh_t: dict = {}
            oc_t: dict = {}

            def load_block(blk):
                t = ohpool.tile([128, ohcols], mybir.dt.float8e4)
                nc.default_dma_engine.dma_start(
                    t[:], oh_d.ap()[:, blk * ohcols:(blk + 1) * ohcols])
                oh_t[blk] = t
                t = ocpool.tile([128, occols], mybir.dt.float8e4)
                nc.default_dma_engine.dma_start(
                    t[:], oc_d.ap()[:, blk * occols:(blk + 1) * occols])
                oc_t[blk] = t

            for blk in range(min(3, NBLK)):
                load_block(blk)

            def stage2(g, ht):
                hts = htpool.tile([VOCAB, GROUP * W], mybir.dt.bfloat16)
                nc.scalar.copy(hts[:], ht[0:VOCAB, :])
                ops = ps_out.tile([128, 2 * HIDDEN], mybir.dt.float32)
                for wp in range(2):
                    lhsT = hts[:, wp * 128:(wp + 1) * 128]
                    o = ops[:, wp * HIDDEN:(wp + 1) * HIDDEN]
                    nc.tensor.matmul(o, lhsT, wmat[:, 0:HIDDEN],
                                     start=True, stop=False)
                    nc.tensor.matmul(o, lhsT, wmat[:, HIDDEN:2 * HIDDEN],
                                     start=False, stop=True)
                outs = outpool.tile([128, 2 * HIDDEN], mybir.dt.float32)
                nc.vector.tensor_copy(outs[:], ops[:])
                dst = out_ap[g * GROUP * W:(g + 1) * GROUP * W, :].rearrange(
                    "(wp b) h -> b wp h", wp=2)
                nc.scalar.dma_start(dst, outs[:].rearrange(
                    "b (wp h) -> b wp h", wp=2))

            pending = None
            for g in range(NWIN // GROUP):
                ht = ps_ht.tile([VC, GROUP * W], mybir.dt.float32)
                for wi in range(GROUP):
                    w = g * GROUP + wi
                    blk, wloc = divmod(w, WPB)
                    if wloc == 0 and blk + 3 < NBLK:
                        load_block(blk + 3)
                    for dc in range(D):
                        j = wloc * D + dc
                        oh3 = oh_t[blk][:, j * 2 * W:(j + 1) * 2 * W].rearrange(
                            "p (two s) -> p two s", two=2)
                        oc3 = oc_t[blk][:, j * 2 * VC:(j + 1) * 2 * VC].rearrange(
                            "p (two v) -> p two v", two=2)
                        nc.tensor.matmul(
                            ht[:, wi * W:(wi + 1) * W], oc3, oh3,
                            start=(dc == 0), stop=(dc == D - 1),
                            perf_mode=mybir.MatmulPerfMode.DoubleRow)
                if pending is not None:
                    stage2(*pending)
                pending = (g, ht)
            stage2(*pending)
    nc.compile()
    return nc


_EYE_OH = np.zeros((W + 1, W), FP8)
_EYE_OH[np.arange(W), np.arange(W)] = 1
_EYE_OC = np.zeros((VOCAB + 1, VC), FP8)
_EYE_OC[np.arange(VOCAB), np.arange(VOCAB)] = 1


def _prep_core(local: np.ndarray, cls: np.ndarray, C: int | None):
    """Window-balance one core's pairs.  Returns wcnt_max when C is None,
    else (oh[128, NCH2*128] fp8, oc[128, NCH2*64] fp8, row_of_local[BPC])."""
    cnt = np.bincount(local, minlength=BPC)
    order = np.argsort(cnt, kind="stable")[::-1]
    r = np.arange(BPC)
    passi, pos = divmod(r, NWIN)
    wser = np.where(passi % 2 == 0, pos, NWIN - 1 - pos)
    w_of_bin = np.empty(BPC, np.int32)
    s_of_bin = np.empty(BPC, np.int32)
    w_of_bin[order] = wser
    s_of_bin[order] = passi
    wkey = w_of_bin[local]
    wcnt = np.bincount(wkey, minlength=NWIN)
    if C is None:
        return int(wcnt.max())

    NCH2 = NWIN * C // 2
    order1 = np.argsort(wkey, kind="stable")
    wsorted = wkey[order1]
    starts = np.zeros(NWIN, np.int64)
    np.cumsum(wcnt[:-1], out=starts[1:])
    idx_in_w = np.arange(len(local)) - starts[wsorted]
    dest = wsorted.astype(np.int64) * (C * 128) + idx_in_w

    slot_pad = np.full(NWIN * C * 128, W, np.int16)
    slot_pad[dest] = s_of_bin[local[order1]]
    cls_pad = np.full(NWIN * C * 128, VOCAB, np.int16)
    cls_pad[dest] = cls[order1]

    oh_in = np.ascontiguousarray(
        _EYE_OH[slot_pad].reshape(NCH2, 2, 128, W).transpose(2, 0, 1, 3)
    ).reshape(128, NCH2 * 2 * W)
    oc_in = np.ascontiguousarray(
        _EYE_OC[cls_pad].reshape(NCH2, 2, 128, VC).transpose(2, 0, 1, 3)
    ).reshape(128, NCH2 * 2 * VC)
    row_of_local = (w_of_bin * W + s_of_bin).astype(np.int64)
    return oh_in, oc_in, row_of_local


def _make_in_maps(x, atom_to_cycle, emb_W, C=None):
    src = np.asarray(atom_to_cycle[0], dtype=np.int64)
    seg = np.asarray(atom_to_cycle[1], dtype=np.int64)
    cls_all = np.asarray(x, dtype=np.int16)[src]

    order0 = np.argsort(seg, kind="stable")
    seg_s = seg[order0]
    cls_s = cls_all[order0]
    bounds = np.searchsorted(seg_s, np.arange(NCORES + 1) * BPC)

    cores = []
    for c in range(NCORES):
        lo, hi = bounds[c], bounds[c + 1]
        cores.append((np.asarray(seg_s[lo:hi] - c * BPC, np.int64),
                      cls_s[lo:hi]))

    if C is None:
        wmax = max(_prep_core(l, k, None) for l, k in cores)
        C = max(6, 2 * (-(-wmax // 256)))

    w32 = np.asarray(emb_W, np.float32)
    w_hi = w32.astype(BF16)
    w_lo = (w32 - w_hi.astype(np.float32)).astype(BF16)
    wmat_in = np.concatenate([w_hi, w_lo], axis=1)

    in_maps, rowmaps = [], []
    for local, k in cores:
        oh_in, oc_in, rowmap = _prep_core(local, k, C)
        in_maps.append({"wmat": wmat_in, "oh": oh_in, "oc": oc_in})
        rowmaps.append(rowmap)
    return C, in_maps, rowmaps


def kernel(x, atom_to_cycle, emb_W, n_cycles):
    assert int(n_cycles) == N_CYCLES
    x = np.asarray(x)
    atom_to_cycle = np.asarray(atom_to_cycle)
    emb_W = np.asarray(emb_W, np.float32)
    assert atom_to_cycle.shape == (2, N_PAIRS) and emb_W.shape == (VOCAB, HIDDEN)

    C, in_maps, rowmaps = _make_in_maps(x, atom_to_cycle, emb_W)
    if C not in _prog_cache:
        _prog_cache[C] = _build_program(C)
    nc = _prog_cache[C]

    res = run_bass_kernel_spmd(nc, in_maps, list(range(NCORES))).results

    out = np.empty((N_CYCLES, HIDDEN), np.float32)
    for c in range(NCORES):
        out[c * BPC:(c + 1) * BPC] = res[c]["out"][rowmaps[c]]
    return out


# revision 5
# speedup vs baseline: 1.8167x; 1.0798x over previous
"""Trainium2 Bass kernel for nn_CycleEmbedding0 (gnn_message_passing).

Computes out = segment_sum(emb_W[x][atom_to_cycle[0]], atom_to_cycle[1], 200000).

Key algebraic reduction: the embedding table has only VOCAB=22 rows, so
    out[c, :] = sum_v H[c, v] * emb_W[v, :]
where H[c, v] = #{pairs p : seg[p] == c and x[src[p]] == v} is a class
histogram.  This cuts memory traffic ~8x vs the naive gather/scatter.

Distribution (8 NeuronCores): cycle bins are range-sharded across cores
(25000 bins/core).  On the host, each core's bins are load-balanced into
392 windows of 64 bin-slots (serpentine assignment by bin popularity) and
the core's pairs are bucketed per window, padded to C chunks of 128.

Device kernel per core (identical SPMD program):
  stage 1 (histogram): per 256-pair double-chunk, TensorE accumulates
    psum_HT[v, slot] += sum_i OC[:,i,:].T @ OH[:,i,:] with fp8 DoubleRow
    matmuls (2 MACs/cell/cycle).  OH (slot one-hots) and OC (class
    one-hots) are built on the host as fp8 and streamed in.
  stage 2 (apply emb): per window-pair, out[128, 128] = HT^T @ (W_hi+W_lo)
    with emb_W split into two bf16 matrices for fp32-level accuracy;
    ScalarE evacuates the histogram, VectorE evacuates the output;
    DMA writes [25088, 128] f32 per core.

Host gathers the 8 core outputs and un-permutes rows back to cycle order.
"""

import numpy as np
import ml_dtypes
from contextlib import ExitStack

import concourse.bass as bass
import concourse.tile as tile
import concourse.mybir as mybir
from concourse import bacc
from concourse.bass_utils import run_bass_kernel_spmd

BF16 = ml_dtypes.bfloat16
FP8 = ml_dtypes.float8_e4m3

N_ATOMS = 500000
N_PAIRS = 2000000
N_CYCLES = 200000
VOCAB = 22
HIDDEN = 128

NCORES = 8
BPC = N_CYCLES // NCORES      # bins (cycles) per core
W = 64                        # bin-slots per window
VC = 32                       # class cols padded (DoubleRow needs step%16==0)
NWIN = 392                    # windows per core
NBLK = 28                     # OH/OC streamed in NBLK blocks
WPB = NWIN // NBLK            # windows per block
GROUP = 4                     # windows per psum group
SMAX = -(-BPC // NWIN)        # max slot index + 1 (<= 64)
assert SMAX <= W and NWIN % GROUP == 0 and NWIN % NBLK == 0

_prog_cache: dict = {}


def _build_program(C: int):
    """One SPMD program; C (even) chunks of 128 pairs per window."""
    assert C % 2 == 0
    D = C // 2                    # double-chunks per window
    NCH2 = NWIN * D               # double-chunks per core
    nc = bacc.Bacc("TRN2", target_bir_lowering=False, debug=False,
                   num_devices=NCORES)
    wmat_d = nc.dram_tensor("wmat", [VOCAB, 2 * HIDDEN], mybir.dt.bfloat16,
                            kind="ExternalInput")
    oh_d = nc.dram_tensor("oh", [128, NCH2 * 2 * W], mybir.dt.float8e4,
                          kind="ExternalInput")
    oc_d = nc.dram_tensor("oc", [128, NCH2 * 2 * VC], mybir.dt.float8e4,
                          kind="ExternalInput")
    out_d = nc.dram_tensor("out", [NWIN * W, HIDDEN], mybir.dt.float32,
                           kind="ExternalOutput")
    out_ap = out_d.ap()
    ohcols = WPB * D * 2 * W      # per-block free dim
    occols = WPB * D * 2 * VC

    with tile.TileContext(nc) as tc:
        with ExitStack() as ctx:
            const = ctx.enter_context(tc.tile_pool(name="const", bufs=1))
            ohpool = ctx.enter_context(tc.tile_pool(name="ohblk", bufs=4))
            ocpool = ctx.enter_context(tc.tile_pool(name="ocblk", bufs=4))
            htpool = ctx.enter_context(tc.tile_pool(name="hts", bufs=3))
            outpool = ctx.enter_context(tc.tile_pool(name="outs", bufs=3))
            ps_ht = ctx.enter_context(
                tc.tile_pool(name="psht", bufs=3, space=bass.MemorySpace.PSUM))
            ps_out = ctx.enter_context(
                tc.tile_pool(name="psout", bufs=2, space=bass.MemorySpace.PSUM))

            wmat = const.tile([VOCAB, 2 * HIDDEN], mybir.dt.bfloat16)
            nc.default_dma_engine.dma_start(wmat[:], wmat_d.ap())

            oh_t: dict = {}
            oc_t: dict = {}

            def load_block(blk):
                t = ohpool.tile([128, ohcols], mybir.dt.float8e4)
                nc.sync.dma_start(
                    t[:], oh_d.ap()[:, blk * ohcols:(blk + 1) * ohcols])
                oh_t[blk] = t
                t = ocpool.tile([128, occols], mybir.dt.float8e4)
                nc.gpsimd.dma_start(
                    t[:], oc_d.ap()[:, blk * occols:(blk + 1) * occols])
                oc_t[blk] = t

            for blk in range(min(3, NBLK)):
                load_block(blk)

            def stage2(g, ht):
                hts = htpool.tile([VOCAB, GROUP * W], mybir.dt.bfloat16)
                nc.scalar.copy(hts[:], ht[0:VOCAB, :])
                ops = ps_out.tile([128, 2 * HIDDEN], mybir.dt.float32)
                for wp in range(2):
                    lhsT = hts[:, wp * 128:(wp + 1) * 128]
                    o = ops[:, wp * HIDDEN:(wp + 1) * HIDDEN]
                    nc.tensor.matmul(o, lhsT, wmat[:, 0:HIDDEN],
                                     start=True, stop=False)
                    nc.tensor.matmul(o, lhsT, wmat[:, HIDDEN:2 * HIDDEN],
                                     start=False, stop=True)
                outs = outpool.tile([128, 2 * HIDDEN], mybir.dt.float32)
                nc.vector.tensor_copy(outs[:], ops[:])
                dst = out_ap[g * GROUP * W:(g + 1) * GROUP * W, :].rearrange(
                    "(wp b) h -> b wp h", wp=2)
                nc.scalar.dma_start(dst, outs[:].rearrange(
                    "b (wp h) -> b wp h", wp=2))

            pending = None
            for g in range(NWIN // GROUP):
                ht = ps_ht.tile([VC, GROUP * W], mybir.dt.float32)
                for wi in range(GROUP):
                    w = g * GROUP + wi
                    blk, wloc = divmod(w, WPB)
                    if wloc == 0 and blk + 3 < NBLK:
                        load_block(blk + 3)
                    for dc in range(D):
                        j = wloc * D + dc
                        oh3 = oh_t[blk][:, j * 2 * W:(j + 1) * 2 * W].rearrange(
                            "p (two s) -> p two s", two=2)
                        oc3 = oc_t[blk][:, j * 2 * VC:(j + 1) * 2 * VC].rearrange(
                            "p (two v) -> p two v", two=2)
                        nc.tensor.matmul(
                            ht[:, wi * W:(wi + 1) * W], oc3, oh3,
                            start=(dc == 0), stop=(dc == D - 1),
                            perf_mode=mybir.MatmulPerfMode.DoubleRow)
                if pending is not None:
                    stage2(*pending)
                pending = (g, ht)
            stage2(*pending)
    nc.compile()
    return nc


_EYE_OH = np.zeros((W + 1, W), FP8)
_EYE_OH[np.arange(W), np.arange(W)] = 1
_EYE_OC = np.zeros((VOCAB + 1, VC), FP8)
_EYE_OC[np.arange(VOCAB), np.arange(VOCAB)] = 1


def _prep_core(local: np.ndarray, cls: np.ndarray, C: int | None):
    """Window-balance one core's pairs.  Returns wcnt_max when C is None,
    else (oh[128, NCH2*128] fp8, oc[128, NCH2*64] fp8, row_of_local[BPC])."""
    cnt = np.bincount(local, minlength=BPC)
    order = np.argsort(cnt, kind="stable")[::-1]
    r = np.arange(BPC)
    passi, pos = divmod(r, NWIN)
    wser = np.where(passi % 2 == 0, pos, NWIN - 1 - pos)
    w_of_bin = np.empty(BPC, np.int32)
    s_of_bin = np.empty(BPC, np.int32)
    w_of_bin[order] = wser
    s_of_bin[order] = passi
    wkey = w_of_bin[local]
    wcnt = np.bincount(wkey, minlength=NWIN)
    if C is None:
        return int(wcnt.max())

    NCH2 = NWIN * C // 2
    order1 = np.argsort(wkey, kind="stable")
    wsorted = wkey[order1]
    starts = np.zeros(NWIN, np.int64)
    np.cumsum(wcnt[:-1], out=starts[1:])
    idx_in_w = np.arange(len(local)) - starts[wsorted]
    dest = wsorted.astype(np.int64) * (C * 128) + idx_in_w

    slot_pad = np.full(NWIN * C * 128, W, np.int16)
    slot_pad[dest] = s_of_bin[local[order1]]
    cls_pad = np.full(NWIN * C * 128, VOCAB, np.int16)
    cls_pad[dest] = cls[order1]

    oh_in = np.ascontiguousarray(
        _EYE_OH[slot_pad].reshape(NCH2, 2, 128, W).transpose(2, 0, 1, 3)
    ).reshape(128, NCH2 * 2 * W)
    oc_in = np.ascontiguousarray(
        _EYE_OC[cls_pad].reshape(NCH2, 2, 128, VC).transpose(2, 0, 1, 3)
    ).reshape(128, NCH2 * 2 * VC)
    row_of_local = (w_of_bin * W + s_of_bin).astype(np.int64)
    return oh_in, oc_in, row_of_local


def _make_in_maps(x, atom_to_cycle, emb_W, C=None):
    src = np.asarray(atom_to_cycle[0], dtype=np.int64)
    seg = np.asarray(atom_to_cycle[1], dtype=np.int64)
    cls_all = np.asarray(x, dtype=np.int16)[src]

    order0 = np.argsort(seg, kind="stable")
    seg_s = seg[order0]
    cls_s = cls_all[order0]
    bounds = np.searchsorted(seg_s, np.arange(NCORES + 1) * BPC)

    cores = []
    for c in range(NCORES):
        lo, hi = bounds[c], bounds[c + 1]
        cores.append((np.asarray(seg_s[lo:hi] - c * BPC, np.int64),
                      cls_s[lo:hi]))

    if C is None:
        wmax = max(_prep_core(l, k, None) for l, k in cores)
        C = max(6, 2 * (-(-wmax // 256)))

    w32 = np.asarray(emb_W, np.float32)
    w_hi = w32.astype(BF16)
    w_lo = (w32 - w_hi.astype(np.float32)).astype(BF16)
    wmat_in = np.concatenate([w_hi, w_lo], axis=1)

    in_maps, rowmaps = [], []
    for local, k in cores:
        oh_in, oc_in, rowmap = _prep_core(local, k, C)
        in_maps.append({"wmat": wmat_in, "oh": oh_in, "oc": oc_in})
        rowmaps.append(rowmap)
    return C, in_maps, rowmaps


def kernel(x, atom_to_cycle, emb_W, n_cycles):
    assert int(n_cycles) == N_CYCLES
    x = np.asarray(x)
    atom_to_cycle = np.asarray(atom_to_cycle)
    emb_W = np.asarray(emb_W, np.float32)
    assert atom_to_cycle.shape == (2, N_PAIRS) and emb_W.shape == (VOCAB, HIDDEN)

    C, in_maps, rowmaps = _make_in_maps(x, atom_to_cycle, emb_W)
    if C not in _prog_cache:
        _prog_cache[C] = _build_program(C)
    nc = _prog_cache[C]

    res = run_bass_kernel_spmd(nc, in_maps, list(range(NCORES))).results

    out = np.empty((N_CYCLES, HIDDEN), np.float32)
    for c in range(NCORES):
        out[c * BPC:(c + 1) * BPC] = res[c]["out"][rowmaps[c]]
    return out


# revision 10
# speedup vs baseline: 2.1059x; 1.1592x over previous
"""Trainium2 Bass kernel for nn_CycleEmbedding0 (gnn_message_passing).

Computes out = segment_sum(emb_W[x][atom_to_cycle[0]], atom_to_cycle[1], 200000).

Key algebraic reduction: the embedding table has only VOCAB=22 rows, so
    out[c, :] = sum_v H[c, v] * emb_W[v, :]
where H[c, v] = #{pairs p : seg[p] == c and x[src[p]] == v} is a class
histogram.  This cuts memory traffic ~8x vs the naive gather/scatter.

Distribution (8 NeuronCores): cycle bins are range-sharded across cores
(25000 bins/core).  On the host, each core's bins are load-balanced into
392 windows of 64 bin-slots (serpentine assignment by bin popularity) and
the core's pairs are bucketed per window, padded to C chunks of 128.

Device kernel per core (identical SPMD program):
  stage 1 (histogram): per 256-pair double-chunk, TensorE accumulates
    psum_HT[v, slot] += sum_i OC[:,i,:].T @ OH[:,i,:] with fp8 DoubleRow
    matmuls (2 MACs/cell/cycle).  OH (slot one-hots) and OC (class
    one-hots) are built on the host as fp8 and streamed in.
  stage 2 (apply emb): per window-pair, out[128, 128] = HT^T @ (W_hi+W_lo)
    with emb_W split into two bf16 matrices for fp32-level accuracy;
    ScalarE evacuates the histogram, VectorE evacuates the output;
    DMA writes [25088, 128] f32 per core.

Host gathers the 8 core outputs and un-permutes rows back to cycle order.
"""

import numpy as np
import ml_dtypes
from contextlib import ExitStack

import concourse.bass as bass
import concourse.tile as tile
import concourse.mybir as mybir
from concourse import bacc
from concourse.bass_utils import run_bass_kernel_spmd

BF16 = ml_dtypes.bfloat16
FP8 = ml_dtypes.float8_e4m3

N_ATOMS = 500000
N_PAIRS = 2000000
N_CYCLES = 200000
VOCAB = 22
HIDDEN = 128

NCORES = 8
BPC = N_CYCLES // NCORES      # bins (cycles) per core
W = 64                        # bin-slots per window
VC = 32                       # class cols padded (DoubleRow needs step%16==0)
NWIN = 392                    # windows per core
NBLK = 28                     # OH/OC streamed in NBLK blocks
WPB = NWIN // NBLK            # windows per block
GROUP = 4                     # windows per psum group
SMAX = -(-BPC // NWIN)        # max slot index + 1 (<= 64)
assert SMAX <= W and NWIN % GROUP == 0 and NWIN % NBLK == 0

_prog_cache: dict = {}


def _build_program(C: int):
    """One SPMD program; C (even) chunks of 128 pairs per window."""
    assert C % 2 == 0
    D = C // 2                    # double-chunks per window
    NCH2 = NWIN * D               # double-chunks per core
    nc = bacc.Bacc("TRN2", target_bir_lowering=False, debug=False,
                   num_devices=NCORES)
    wmat_d = nc.dram_tensor("wmat", [VOCAB, 2 * HIDDEN], mybir.dt.bfloat16,
                            kind="ExternalInput")
    oh_d = nc.dram_tensor("oh", [128, NCH2 * 2 * W], mybir.dt.float8e4,
                          kind="ExternalInput")
    oc_d = nc.dram_tensor("oc", [128, NCH2 * 2 * VC], mybir.dt.float8e4,
                          kind="ExternalInput")
    out_d = nc.dram_tensor("out", [NWIN * W, HIDDEN], mybir.dt.float32,
                           kind="ExternalOutput")
    out_ap = out_d.ap()
    ohcols = WPB * D * 2 * W      # per-block free dim
    occols = WPB * D * 2 * VC

    with tile.TileContext(nc) as tc:
        with ExitStack() as ctx:
            const = ctx.enter_context(tc.tile_pool(name="const", bufs=1))
            ohpool = ctx.enter_context(tc.tile_pool(name="ohblk", bufs=4))
            ocpool = ctx.enter_context(tc.tile_pool(name="ocblk", bufs=4))
            htpool = ctx.enter_context(tc.tile_pool(name="hts", bufs=3))
            outpool = ctx.enter_context(tc.tile_pool(name="outs", bufs=3))
            ps_ht = ctx.enter_context(
                tc.tile_pool(name="psht", bufs=3, space=bass.MemorySpace.PSUM))
            ps_out = ctx.enter_context(
                tc.tile_pool(name="psout", bufs=2, space=bass.MemorySpace.PSUM))

            wmat = const.tile([VOCAB, 2 * HIDDEN], mybir.dt.bfloat16)
            nc.default_dma_engine.dma_start(wmat[:], wmat_d.ap())

            oh_t: dict = {}
            oc_t: dict = {}

            def load_block(blk):
                t = ohpool.tile([128, ohcols], mybir.dt.float8e4)
                oh_eng = nc.scalar if blk % 6 == 5 else nc.sync
                oh_eng.dma_start(
                    t[:], oh_d.ap()[:, blk * ohcols:(blk + 1) * ohcols])
                oh_t[blk] = t
                t = ocpool.tile([128, occols], mybir.dt.float8e4)
                nc.gpsimd.dma_start(
                    t[:], oc_d.ap()[:, blk * occols:(blk + 1) * occols])
                oc_t[blk] = t

            for blk in range(min(3, NBLK)):
                load_block(blk)

            outs_box = [None]

            def stage2(g, ht):
                hts = htpool.tile([VOCAB, GROUP * W], mybir.dt.bfloat16)
                nc.scalar.copy(hts[:], ht[0:VOCAB, :])
                ops = ps_out.tile([128, 2 * HIDDEN], mybir.dt.float32)
                for wp in range(2):
                    lhsT = hts[:, wp * 128:(wp + 1) * 128]
                    o = ops[:, wp * HIDDEN:(wp + 1) * HIDDEN]
                    nc.tensor.matmul(o, lhsT, wmat[:, 0:HIDDEN],
                                     start=True, stop=False)
                    nc.tensor.matmul(o, lhsT, wmat[:, HIDDEN:2 * HIDDEN],
                                     start=False, stop=True)
                # batch 2 groups per SBUF tile so the out-DMA moves 2 KiB
                # per partition row instead of 512 B
                half = g % 2
                if half == 0:
                    outs_box[0] = outpool.tile([128, 4 * HIDDEN],
                                               mybir.dt.float32, name="outs", tag="outs")
                outs = outs_box[0]
                nc.vector.tensor_copy(
                    outs[:, half * 2 * HIDDEN:(half + 1) * 2 * HIDDEN], ops[:])
                if half == 1:
                    g0 = g - 1
                    dst = out_ap[g0 * GROUP * W:(g0 + 2) * GROUP * W,
                                 :].rearrange("(wp b) h -> b wp h", wp=4)
                    nc.scalar.dma_start(dst, outs[:].rearrange(
                        "b (wp h) -> b wp h", wp=4))

            pending = None
            for g in range(NWIN // GROUP):
                ht = ps_ht.tile([VC, GROUP * W], mybir.dt.float32)
                for wi in range(GROUP):
                    w = g * GROUP + wi
                    blk, wloc = divmod(w, WPB)
                    if wloc == 0 and blk + 3 < NBLK:
                        load_block(blk + 3)
                    for dc in range(D):
                        j = wloc * D + dc
                        oh3 = oh_t[blk][:, j * 2 * W:(j + 1) * 2 * W].rearrange(
                            "p (two s) -> p two s", two=2)
                        oc3 = oc_t[blk][:, j * 2 * VC:(j + 1) * 2 * VC].rearrange(
                            "p (two v) -> p two v", two=2)
                        nc.tensor.matmul(
                            ht[:, wi * W:(wi + 1) * W], oc3, oh3,
                            start=(dc == 0), stop=(dc == D - 1),
                            perf_mode=mybir.MatmulPerfMode.DoubleRow)
                if pending is not None:
                    stage2(*pending)
                pending = (g, ht)
            stage2(*pending)
    nc.compile()
    return nc


_EYE_OH = np.zeros((W + 1, W), FP8)
_EYE_OH[np.arange(W), np.arange(W)] = 1
_EYE_OC = np.zeros((VOCAB + 1, VC), FP8)
_EYE_OC[np.arange(VOCAB), np.arange(VOCAB)] = 1


def _prep_core(local: np.ndarray, cls: np.ndarray, C: int | None):
    """Window-balance one core's pairs.  Returns wcnt_max when C is None,
    else (oh[128, NCH2*128] fp8, oc[128, NCH2*64] fp8, row_of_local[BPC])."""
    cnt = np.bincount(local, minlength=BPC)
    order = np.argsort(cnt, kind="stable")[::-1]
    r = np.arange(BPC)
    passi, pos = divmod(r, NWIN)
    wser = np.where(passi % 2 == 0, pos, NWIN - 1 - pos)
    w_of_bin = np.empty(BPC, np.int32)
    s_of_bin = np.empty(BPC, np.int32)
    w_of_bin[order] = wser
    s_of_bin[order] = passi
    wkey = w_of_bin[local]
    wcnt = np.bincount(wkey, minlength=NWIN)
    if C is None:
        return int(wcnt.max())

    NCH2 = NWIN * C // 2
    order1 = np.argsort(wkey, kind="stable")
    wsorted = wkey[order1]
    starts = np.zeros(NWIN, np.int64)
    np.cumsum(wcnt[:-1], out=starts[1:])
    idx_in_w = np.arange(len(local)) - starts[wsorted]
    dest = wsorted.astype(np.int64) * (C * 128) + idx_in_w

    slot_pad = np.full(NWIN * C * 128, W, np.int16)
    slot_pad[dest] = s_of_bin[local[order1]]
    cls_pad = np.full(NWIN * C * 128, VOCAB, np.int16)
    cls_pad[dest] = cls[order1]

    oh_in = np.ascontiguousarray(
        _EYE_OH[slot_pad].reshape(NCH2, 2, 128, W).transpose(2, 0, 1, 3)
    ).reshape(128, NCH2 * 2 * W)
    oc_in = np.ascontiguousarray(
        _EYE_OC[cls_pad].reshape(NCH2, 2, 128, VC).transpose(2, 0, 1, 3)
    ).reshape(128, NCH2 * 2 * VC)
    row_of_local = (w_of_bin * W + s_of_bin).astype(np.int64)
    return oh_in, oc_in, row_of_local


def _make_in_maps(x, atom_to_cycle, emb_W, C=None):
    src = np.asarray(atom_to_cycle[0], dtype=np.int64)
    seg = np.asarray(atom_to_cycle[1], dtype=np.int64)
    cls_all = np.asarray(x, dtype=np.int16)[src]

    order0 = np.argsort(seg, kind="stable")
    seg_s = seg[order0]
    cls_s = cls_all[order0]
    bounds = np.searchsorted(seg_s, np.arange(NCORES + 1) * BPC)

    cores = []
    for c in range(NCORES):
        lo, hi = bounds[c], bounds[c + 1]
        cores.append((np.asarray(seg_s[lo:hi] - c * BPC, np.int64),
                      cls_s[lo:hi]))

    if C is None:
        wmax = max(_prep_core(l, k, None) for l, k in cores)
        C = max(6, 2 * (-(-wmax // 256)))

    w32 = np.asarray(emb_W, np.float32)
    w_hi = w32.astype(BF16)
    w_lo = (w32 - w_hi.astype(np.float32)).astype(BF16)
    wmat_in = np.concatenate([w_hi, w_lo], axis=1)

    in_maps, rowmaps = [], []
    for local, k in cores:
        oh_in, oc_in, rowmap = _prep_core(local, k, C)
        in_maps.append({"wmat": wmat_in, "oh": oh_in, "oc": oc_in})
        rowmaps.append(rowmap)
    return C, in_maps, rowmaps


def kernel(x, atom_to_cycle, emb_W, n_cycles):
    assert int(n_cycles) == N_CYCLES
    x = np.asarray(x)
    atom_to_cycle = np.asarray(atom_to_cycle)
    emb_W = np.asarray(emb_W, np.float32)
    assert atom_to_cycle.shape == (2, N_PAIRS) and emb_W.shape == (VOCAB, HIDDEN)

    C, in_maps, rowmaps = _make_in_maps(x, atom_to_cycle, emb_W)
    if C not in _prog_cache:
        _prog_cache[C] = _build_program(C)
    nc = _prog_cache[C]

    res = run_bass_kernel_spmd(nc, in_maps, list(range(NCORES))).results

    out = np.empty((N_CYCLES, HIDDEN), np.float32)
    for c in range(NCORES):
        out[c * BPC:(c + 1) * BPC] = res[c]["out"][rowmaps[c]]
    return out
